# revision 12
# baseline (speedup 1.0000x reference)
"""GCN message-passing kernel for Trainium2, 8-core SPMD.

Strategy:
 - Nodes sharded contiguously across 8 cores (6250/core, padded to 6272 = 49*128).
 - Per layer: each core computes its shard of the "message table"
   t = dis * (h @ W + b) in fp16, AllGather -> full table in DRAM.
 - Symmetric norm is separable: norm(e) = dis[src]*dis[dst], so messages are
   gathered from the pre-scaled table and summed un-weighted; the dst factor is
   applied after aggregation.  Self loops are injected as ordinary edges.
 - Edges (incl. self loops) are grouped by dst into 128-node groups, sorted by
   dst, tiled into columns of 128 edges.  Each column is aggregated with one
   PE matmul: out[d, f] += sum_e S[e, d] * msg[e, f], where S is a one-hot
   (edge -> dst-within-window) matrix built on-device with iota + is_equal.
 - Gathers use dma_gather with int16 indices; the table is split in two halves
   (lo/hi) so indices stay < 32768.
 - h_next = relu(dis * agg) via the scalar engine with per-partition scale.
"""
import sys
import os
import numpy as np

for _p in ("/opt/trn_rl_repo",):
    if _p not in sys.path and os.path.isdir(_p):
        sys.path.insert(0, _p)

P = 128          # partitions
W = 64           # dst window per S matrix
F = 128          # features
C = 40           # classes
NCORES = 8
CHUNK = 5        # groups per gather chunk


def _ceil_div(a, b):
    return -(-a // b)


def _layout_cols(d_lists, base_override=None):
    """Shared column layout for one (group, half) edge list: number of columns
    (same for all cores) and a window base per column such that every core's
    column-j edges fit in [base_j, base_j + W)."""
    cnts = [len(d) for d in d_lists]
    mx = max(cnts)
    if mx == 0:
        return [], True
    K = _ceil_div(mx, P)
    bases = []
    for j in range(K):
        lo = None
        hi = None
        for d in d_lists:
            if len(d) > j * P:
                a = d[j * P]
                b = d[min((j + 1) * P, len(d)) - 1]
                lo = a if lo is None else min(lo, a)
                hi = b if hi is None else max(hi, b)
        if base_override is not None:
            base = base_override
        else:
            base = max(0, min((int(lo) // 32) * 32, P - W))
        if hi - base >= W:
            return None, False
        bases.append(base)
    return bases, True


def _prep_host(x, edge_index, E_meta, w0, b0, gcn_W, gcn_b, lt1_W, lt1_b, N):
    NPC_REAL = N // NCORES
    G = _ceil_div(NPC_REAL, P)          # groups per core
    NPC = G * P                          # padded nodes per core
    TROWS = NCORES * NPC
    THALF = TROWS // 2
    assert THALF < 32768

    src = np.ascontiguousarray(edge_index[0]).astype(np.int64)
    dst = np.ascontiguousarray(edge_index[1]).astype(np.int64)

    deg = np.bincount(dst, minlength=N).astype(np.float32) + 1.0
    dis = deg ** -0.5

    loops = np.arange(N, dtype=np.int64)
    src2 = np.concatenate([src, loops])
    dst2 = np.concatenate([dst, loops])

    new_w0 = np.maximum(
        w0.astype(np.float32) @ E_meta.astype(np.float32) + b0.astype(np.float32), 0.0)
    W0p = (new_w0 @ np.asarray(gcn_W[0], np.float32)).astype(np.float32)

    tr = NPC * (src2 // NPC_REAL) + (src2 % NPC_REAL)
    half = (tr >= THALF).astype(np.int64)
    rloc = tr - THALF * half

    core = dst2 // NPC_REAL
    dl = dst2 - core * NPC_REAL
    g = dl // P
    d128 = dl - g * P

    order = np.lexsort((dst2, half))
    s_half = half[order]
    s_core = core[order]
    s_g = g[order]
    s_d = d128[order]
    s_r = rloc[order]

    key = (s_half * NCORES + s_core) * G + s_g
    nkeys = 2 * NCORES * G
    starts = np.searchsorted(key, np.arange(nkeys))
    ends = np.searchsorted(key, np.arange(nkeys) + 1)

    def seg(h, c, gg):
        k = (h * NCORES + c) * G + gg
        return starts[k], ends[k]

    n_chunks = _ceil_div(G, CHUNK)
    cols_meta = []                 # (g, base, half)
    calls = []                     # (col_start, ncols, half)
    percore_d = [[] for _ in range(NCORES)]
    percore_r = [[] for _ in range(NCORES)]

    for ch in range(n_chunks):
        g_lo = ch * CHUNK
        g_hi = min(G, g_lo + CHUNK)
        for h in (0, 1):
            call_start = len(cols_meta)
            for gg in range(g_lo, g_hi):
                d_lists = []
                r_lists = []
                for c in range(NCORES):
                    a, b = seg(h, c, gg)
                    d_lists.append(s_d[a:b])
                    r_lists.append(s_r[a:b])
                # PE psum outputs must sit in an aligned 64-partition half:
                # split each group's edges by dst value at 64.
                subsets = []
                for lo_v, b_ov in ((0, 0), (W, P - W)):
                    dsub, rsub = [], []
                    for c in range(NCORES):
                        m = (d_lists[c] >= lo_v) & (d_lists[c] < lo_v + W)
                        dsub.append(d_lists[c][m])
                        rsub.append(r_lists[c][m])
                    bs, ok2 = _layout_cols(dsub, base_override=b_ov)
                    assert ok2
                    subsets.append((dsub, rsub, bs))
                for dl_, rl_, bases_ in subsets:
                    for j in range(len(bases_)):
                        cols_meta.append((gg, bases_[j], h))
                        for c in range(NCORES):
                            dcol = np.full(P, -1, np.int64)
                            rcol = np.zeros(P, np.int64)
                            a = j * P
                            b = min((j + 1) * P, len(dl_[c]))
                            if b > a:
                                dcol[: b - a] = dl_[c][a:b] - bases_[j]
                                rcol[: b - a] = rl_[c][a:b]
                            percore_d[c].append(dcol)
                            percore_r[c].append(rcol)
            ncols = len(cols_meta) - call_start
            calls.append((call_start, ncols, h))

    NCOLS = len(cols_meta)
    idx16 = np.zeros((NCORES, P, NCOLS * 8), np.int16)
    dstw = np.zeros((NCORES, P, NCOLS), np.int16)
    for c in range(NCORES):
        r_all = np.concatenate(percore_r[c]) if NCOLS else np.zeros(0, np.int64)
        d_all = np.stack(percore_d[c], axis=1) if NCOLS else np.zeros((P, 0), np.int64)
        ii = np.arange(NCOLS * P)
        a16 = np.zeros((16, NCOLS * 8), np.int16)
        a16[ii % 16, ii // 16] = r_all.astype(np.int16)
        idx16[c] = np.tile(a16, (8, 1))
        dstw[c] = d_all.astype(np.int16)

    xs = np.zeros((NCORES, NPC, F), np.float32)
    diss = np.zeros((NCORES, P, G), np.float32)
    xf = np.asarray(x, np.float32)
    for c in range(NCORES):
        n0 = c * NPC_REAL
        xs[c, :NPC_REAL] = xf[n0:n0 + NPC_REAL]
        dpad = np.zeros(NPC, np.float32)
        dpad[:NPC_REAL] = dis[n0:n0 + NPC_REAL]
        diss[c] = dpad.reshape(G, P).T

    gcn_b = np.asarray(gcn_b, np.float32)
    lt1_b = np.asarray(lt1_b, np.float32)
    biases = [np.tile(gcn_b[l][None, :], (P, 1)) for l in range(3)]
    biases.append(np.tile(lt1_b[None, :], (P, 1)))
    has_b = [bool(np.any(b != 0)) for b in biases]

    meta = dict(N=N, NPC_REAL=NPC_REAL, G=G, NPC=NPC, TROWS=TROWS, THALF=THALF,
                NCOLS=NCOLS, cols_meta=cols_meta, calls=calls, has_b=has_b)
    arrays = dict(idx16=idx16, dstw=dstw, xs=xs, diss=diss, W0p=W0p,
                  W1=np.asarray(gcn_W[1], np.float32),
                  W2=np.asarray(gcn_W[2], np.float32),
                  lt1_W=np.asarray(lt1_W, np.float32),
                  biases=biases)
    return meta, arrays


def _build_program(meta):
    import contextlib
    from concourse import bacc, tile, mybir
    from concourse.masks import make_identity

    G = meta["G"]
    NPC = meta["NPC"]
    TROWS = meta["TROWS"]
    THALF = meta["THALF"]
    NCOLS = meta["NCOLS"]
    cols_meta = meta["cols_meta"]
    calls = meta["calls"]
    has_b = meta["has_b"]

    f32 = mybir.dt.float32
    f16 = mybir.dt.float16
    i16 = mybir.dt.int16
    AF = mybir.ActivationFunctionType

    nc = bacc.Bacc("TRN2", target_bir_lowering=False, debug=False,
                   num_devices=NCORES)

    x_in = nc.dram_tensor("x_in", [NPC, F], f32, kind="ExternalInput")
    dis_in = nc.dram_tensor("dis_in", [P, G], f32, kind="ExternalInput")
    idx_in = nc.dram_tensor("idx_in", [P, NCOLS * 8], i16, kind="ExternalInput")
    dstw_in = nc.dram_tensor("dstw_in", [P, NCOLS], i16, kind="ExternalInput")
    w_in = {
        "W0": nc.dram_tensor("W0_in", [F, F], f32, kind="ExternalInput"),
        "W1": nc.dram_tensor("W1_in", [F, F], f32, kind="ExternalInput"),
        "W2": nc.dram_tensor("W2_in", [F, F], f32, kind="ExternalInput"),
        "WL": nc.dram_tensor("WL_in", [F, C], f32, kind="ExternalInput"),
    }
    b_in = {}
    for l in range(4):
        if has_b[l]:
            cc = F if l < 3 else C
            b_in[l] = nc.dram_tensor(f"B{l}_in", [P, cc], f32, kind="ExternalInput")
    y_out = nc.dram_tensor("y_out", [NPC, C], f32, kind="ExternalOutput")

    with tile.TileContext(nc) as tc:
        with contextlib.ExitStack() as ctx:
            const = ctx.enter_context(tc.tile_pool(name="const", bufs=1))
            big = ctx.enter_context(tc.tile_pool(name="big", bufs=1))
            sh_pool = ctx.enter_context(tc.tile_pool(name="shards", bufs=2))
            msg_pool = ctx.enter_context(tc.tile_pool(name="msgs", bufs=3))
            s_pool = ctx.enter_context(tc.tile_pool(name="smat", bufs=3))
            work = ctx.enter_context(tc.tile_pool(name="work", bufs=4))
            psum = ctx.enter_context(tc.tile_pool(name="psum", bufs=1, space="PSUM"))
            dram = ctx.enter_context(tc.tile_pool(name="dram", bufs=2, space="DRAM"))

            # ---- constants ----
            ident = const.tile([P, P], f32, tag="ident")
            make_identity(nc, ident[:])
            iota64 = const.tile([P, W], i16, tag="iota64")
            nc.gpsimd.iota(iota64[:], pattern=[[1, W]], base=0, channel_multiplier=0)
            zl = const.tile([1, P], f16, tag="zl")
            nc.vector.memset(zl[:], 0)
            zr = const.tile([1, P], f16, tag="zr")
            nc.vector.memset(zr[:], 0)
            dis_t = const.tile([P, G], f32, tag="dis")
            nc.sync.dma_start(dis_t[:], dis_in.ap())
            idx_t = const.tile([P, NCOLS * 8], i16, tag="idx")
            nc.sync.dma_start(idx_t[:], idx_in.ap())
            dstw_t = const.tile([P, NCOLS], i16, tag="dstw")
            nc.sync.dma_start(dstw_t[:], dstw_in.ap())
            w_t = {}
            for nm, cc in (("W0", F), ("W1", F), ("W2", F), ("WL", C)):
                w_t[nm] = const.tile([P, cc], f32, tag=nm, name=f"w_{nm}")
                nc.sync.dma_start(w_t[nm][:], w_in[nm].ap())
            b_t = {}
            for l, t_in in b_in.items():
                cc = F if l < 3 else C
                b_t[l] = const.tile([P, cc], f32, tag=f"B{l}", name=f"b_{l}")
                nc.sync.dma_start(b_t[l][:], t_in.ap())

            shard_dram = [dram.tile([NPC, F], f16, tag="shard_dram", name=f"shard_dram_{i}") for i in range(3)]
            table_dram = [dram.tile([TROWS, F], f16, tag="table_dram",
                                    addr_space="Shared", name=f"table_dram_{i}") for i in range(3)]

            def pass2_group(l, gg, src_t, shard_sb, y_sb):
                """node-major activation tile -> next-table shard slice (l<3)
                or log-softmax logits slice (l==3)."""
                wname = ("W0", "W1", "W2", "WL")[l]
                ncols_o = F if l < 3 else C
                tp = psum.tile([P, P], f32, tag="tp", bufs=1)
                nc.tensor.transpose(out=tp[:], in_=src_t[:], identity=ident[:])
                hT = work.tile([P, P], f32, tag="hT")
                nc.scalar.copy(hT[:], tp[:])
                mm = psum.tile([P, ncols_o], f32, tag="mm", bufs=2)
                nc.tensor.matmul(mm[:], lhsT=hT[:], rhs=w_t[wname][:, :ncols_o],
                                 start=True, stop=True)
                pre = mm
                if has_b[l]:
                    badd = work.tile([P, ncols_o], f32, tag="badd")
                    nc.vector.tensor_add(badd[:], mm[:], b_t[l][:])
                    pre = badd
                if l < 3:
                    nc.scalar.activation(shard_sb[:, gg, :], pre[:], AF.Copy,
                                         scale=dis_t[:, gg:gg + 1])
                else:
                    lg = work.tile([P, C], f32, tag="lg")
                    nc.vector.tensor_copy(lg[:], pre[:])
                    nmax = work.tile([P, 1], f32, tag="nmax")
                    nc.vector.tensor_reduce(nmax[:], lg[:], mybir.AxisListType.X,
                                            mybir.AluOpType.max, negate=True)
                    ex = work.tile([P, C], f32, tag="ex")
                    sm = work.tile([P, 1], f32, tag="sm")
                    nc.scalar.activation(ex[:], lg[:], AF.Exp,
                                         bias=nmax[:, :1], scale=1.0,
                                         accum_out=sm[:, :1])
                    lse = work.tile([P, 1], f32, tag="lse")
                    nc.scalar.activation(lse[:], sm[:], AF.Ln)
                    tot = work.tile([P, 1], f32, tag="tot")
                    nc.vector.tensor_sub(tot[:], nmax[:], lse[:])
                    nc.vector.tensor_scalar_add(y_sb[:, gg, :], lg[:], tot[:, :1])

            def pass2_finish(l, shard_sb, y_sb):
                if l < 3:
                    nc.sync.dma_start(
                        shard_dram[l][:].rearrange("(g p) f -> p g f", p=P),
                        shard_sb[:])
                    if K_NOCC:
                        nc.sync.dma_start(table_dram[l][0:NPC, :], shard_dram[l][:])
                    else:
                        nc.gpsimd.collective_compute(
                            "AllGather", mybir.AluOpType.bypass,
                            replica_groups=[list(range(NCORES))],
                            ins=[shard_dram[l].opt()], outs=[table_dram[l].opt()],
                        )
                else:
                    nc.sync.dma_start(
                        y_out.ap().rearrange("(g p) f -> p g f", p=P), y_sb[:])

            K_NL = int(os.environ.get("K_NL", "3"))
            K_NOCC = bool(int(os.environ.get("K_NOCC", "0")))
            K_NOGATHER = bool(int(os.environ.get("K_NOGATHER", "0")))
            K_NOS = bool(int(os.environ.get("K_NOS", "0")))
            # ---- pre-pass: layer-0 table from x ----
            shard_sb = sh_pool.tile([P, G, F], f16, tag="shard_sb")
            for gg in range(G):
                xt = work.tile([P, F], f32, tag="xt")
                nc.sync.dma_start(xt[:], x_in.ap()[gg * P:(gg + 1) * P, :])
                pass2_group(0, gg, xt, shard_sb, None)
            pass2_finish(0, shard_sb, None)

            # ---- layers ----
            col_of_group = {}
            for ci, (gg, base, h) in enumerate(cols_meta):
                col_of_group.setdefault(gg, []).append(ci)
            last_col = {gg: cols[-1] for gg, cols in col_of_group.items()}
            first_col = {gg: cols[0] for gg, cols in col_of_group.items()}

            for l in range(K_NL):
                tbl = table_dram[l]
                agg = {}
                if l < 2:
                    shard_sb = sh_pool.tile([P, G, F], f16, tag="shard_sb")
                    y_sb = None
                else:
                    shard_sb = None
                    y_sb = big.tile([P, G, C], f32, tag="y_sb")
                for (cs, ncols, h) in calls:
                    if ncols == 0:
                        continue
                    msgs = msg_pool.tile([P, ncols, F], f16, tag="msgs")
                    src_ap = tbl[0:THALF, :] if h == 0 else tbl[THALF:TROWS, :]
                    if K_NOGATHER:
                        nc.vector.memset(msgs[:], 0)
                    elif bool(int(os.environ.get("K_IDXCOPY", "0"))):
                        idx_call = s_pool.tile([P, ncols * 8], i16, tag="idx_call",
                                               name=f"idxc_{l}_{cs}")
                        nc.vector.tensor_copy(idx_call[:], idx_t[:, cs * 8:(cs + ncols) * 8])
                        nc.gpsimd.dma_gather(
                            msgs[:], src_ap, idx_call[:],
                            ncols * P, ncols * P, elem_size=F,
                            single_packet=False)
                    else:
                        nc.gpsimd.dma_gather(
                            msgs[:], src_ap, idx_t[:, cs * 8:(cs + ncols) * 8],
                            ncols * P, ncols * P, elem_size=F,
                            single_packet=False)
                    smat = s_pool.tile([P, ncols, W], f16, tag="smat")
                    if K_NOS:
                        nc.vector.memset(smat[:], 0)
                    elif True:
                        nc.vector.tensor_tensor(
                        out=smat[:],
                        in0=dstw_t[:, cs:cs + ncols, None].to_broadcast([P, ncols, W]),
                        in1=iota64[:, None, :].to_broadcast([P, ncols, W]),
                        op=mybir.AluOpType.is_equal)
                    for j in range(ncols):
                        ci = cs + j
                        gg, base, _h = cols_meta[ci]
                        if ci == first_col[gg]:
                            agg[gg] = psum.tile([P, F], f32, tag="agg", bufs=5, name=f"agg_{gg}")
                            nc.tensor.matmul(agg[gg][:], lhsT=zl[:], rhs=zr[:],
                                             start=True, stop=False,
                                             skip_group_check=True)
                        nc.tensor.matmul(
                            agg[gg][base:base + W, :],
                            lhsT=smat[:, j, :], rhs=msgs[:, j, :],
                            start=False, stop=(ci == last_col[gg]),
                            skip_group_check=True)
                        if ci == last_col[gg]:
                            ht = work.tile([P, F], f32, tag="ht")
                            nc.scalar.activation(ht[:], agg[gg][:], AF.Relu,
                                                 scale=dis_t[:, gg:gg + 1])
                            del agg[gg]
                            pass2_group(l + 1, gg, ht, shard_sb, y_sb)
                pass2_finish(l + 1, shard_sb, y_sb)
            if K_NL < 3:
                y_dummy = big.tile([P, G, C], f32, tag="y_sb", name="y_dummy")
                nc.vector.memset(y_dummy[:], 0)
                nc.sync.dma_start(
                    y_out.ap().rearrange("(g p) f -> p g f", p=P), y_dummy[:])

    nc.compile()
    return nc


def _install_profile_hook():
    """Register the NTFF profiling hook that the agent image leaves out."""
    try:
        import types
        try:
            import antenv.axon_hooks as ah
        except ImportError:
            import antenv
            ah = types.ModuleType("antenv.axon_hooks")
            ah._hook = None
            def _set(h, _m=ah):
                _m._hook = h
            def _get(_m=ah):
                return _m._hook
            ah.set_axon_ntff_profile_hook = _set
            ah.get_axon_ntff_profile_hook = _get
            sys.modules["antenv.axon_hooks"] = ah
            antenv.axon_hooks = ah
        if ah.get_axon_ntff_profile_hook() is not None:
            return
        sys.path.insert(0, "/root/.axon_site/trn_agent_boot")
        import trn_boot
        ah.set_axon_ntff_profile_hook(
            trn_boot._ntff_profile_via_ctypes("/opt/axon/libaxon_pjrt.so"))
        from concourse import bass_utils as bu
        bu.upload_artifacts = lambda tmpdir: tmpdir
    except Exception as e:
        print(f"profile hook install failed: {e}")


def kernel(x, edge_index, E_meta, w0, b0, gcn_W, gcn_b, lt1_W, lt1_b,
           _n_nodes=None, _trace=False, _tmpdir=None):
    from concourse import bass_utils

    N = x.shape[0] if _n_nodes is None else _n_nodes
    meta, arrays = _prep_host(x, edge_index, E_meta, w0, b0, gcn_W, gcn_b,
                              lt1_W, lt1_b, N)
    nc = _build_program(meta)

    in_maps = []
    for c in range(NCORES):
        m = {
            "x_in": arrays["xs"][c],
            "dis_in": arrays["diss"][c],
            "idx_in": arrays["idx16"][c],
            "dstw_in": arrays["dstw"][c],
            "W0_in": arrays["W0p"],
            "W1_in": arrays["W1"],
            "W2_in": arrays["W2"],
            "WL_in": arrays["lt1_W"],
        }
        for l in range(4):
            if meta["has_b"][l]:
                m[f"B{l}_in"] = arrays["biases"][l]
        in_maps.append(m)

    if _trace:
        _install_profile_hook()
    res = bass_utils.run_bass_kernel_spmd(
        nc, in_maps, core_ids=list(range(NCORES)),
        trace=_trace, tmpdir=_tmpdir)

    NPC_REAL = meta["NPC_REAL"]
    out = np.zeros((N, C), np.float32)
    for c in range(NCORES):
        out[c * NPC_REAL:(c + 1) * NPC_REAL] = res.results[c]["y_out"][:NPC_REAL]
    kernel._last_exec_time_ns = res.exec_time_ns
    kernel._last_results = res
    return out
